# revision 8
# baseline (speedup 1.0000x reference)
"""Trainium2 Bass kernel for nn_BiEncoder_63024350101542 (segment_reduce).

Computes, per batch row b of vector_all [B=64, L=512, D=1024]:
    mask[b,j] = (j > first_idx(ids[b]==1)) & (j < first_idx(ids[b]==2))
    span_max  = max over masked rows (fallback: CLS row 0 when mask empty)
    out[b]    = cls + mu * span_max

Sharding: pure data parallelism over the batch dim — 8 batches per
NeuronCore across 8 cores. Each core streams its 16 MiB shard of
vector_all once (memory-bound), doing the masked max on-chip.

Note: every PE (transpose) instruction must carry at most one semaphore
wait — walrus rejects matmuls with multiple embedded waits. All PE
inputs are therefore produced by the vector engine (single DVE sem).
"""

import os
import sys

import numpy as np

for _p in ("/root/.axon_site/_ro/trn_rl_repo", "/opt/trn_rl_repo"):
    if _p not in sys.path and os.path.isdir(_p):
        sys.path.append(_p)

import concourse.bacc as bacc
import concourse.bass as bass
import concourse.mybir as mybir
import concourse.tile as tile
from concourse.bass_utils import run_bass_kernel_spmd

F32 = mybir.dt.float32
I32 = mybir.dt.int32
X = mybir.AxisListType.X
Alu = mybir.AluOpType
Act = mybir.ActivationFunctionType

B, L, D = 64, 512, 1024
NCORES = 8
BPC = B // NCORES          # batches per core
KL = L // 128              # L-tiles per batch (4)
JD = D // 128              # d-blocks (8)
BIG = 1.0e30


def build_bass():
    nc = bacc.Bacc("TRN2", target_bir_lowering=False, debug=False)

    va = nc.dram_tensor("vector_all", [BPC, L, D], F32, kind="ExternalInput").ap()
    ids = nc.dram_tensor("ids", [BPC, L], I32, kind="ExternalInput").ap()
    mu = nc.dram_tensor("mu", [128, 1], F32, kind="ExternalInput").ap()
    iota = nc.dram_tensor("iota", [BPC, L], F32, kind="ExternalInput").ap()
    ident = nc.dram_tensor("identity", [128, 128], F32, kind="ExternalInput").ap()
    out = nc.dram_tensor("out", [BPC, D], F32, kind="ExternalOutput").ap()

    with tile.TileContext(nc) as tc:
        with (
            tc.tile_pool(name="persist", bufs=1) as pp,
            tc.tile_pool(name="xin", bufs=3) as xpool,
            tc.tile_pool(name="masked", bufs=2) as mpool,
            tc.tile_pool(name="red", bufs=2) as rpool,
            tc.tile_pool(name="vout", bufs=2) as vpool,
            tc.tile_pool(name="tr", bufs=4, space="PSUM") as ppool,
            tc.tile_pool(name="smallp", bufs=2, space="PSUM") as spsum,
        ):
            # ---- constants / inputs for the mask stage ----
            ident_in = pp.tile([128, 128], F32)
            nc.sync.dma_start(out=ident_in[:], in_=ident)
            # PE reads only DVE-produced tiles (single-wait rule)
            ident_sb = pp.tile([128, 128], F32)
            nc.vector.tensor_copy(ident_sb[:], ident_in[:])
            mu_col = pp.tile([128, 1], F32)
            nc.sync.dma_start(out=mu_col[:], in_=mu)
            ids_sb = pp.tile([BPC, L], I32)
            nc.sync.dma_start(out=ids_sb[:], in_=ids)
            iota_sb = pp.tile([BPC, L], F32)
            nc.sync.dma_start(out=iota_sb[:], in_=iota)

            # CLS rows gathered into column layout [128, b, j] once
            cls_all = pp.tile([128, BPC, JD], F32)
            for j in range(JD):
                nc.sync.dma_start(
                    out=cls_all[:, :, j],
                    in_=va[:, 0, bass.ts(j, 128)].rearrange("b p -> p b"),
                )

            # ---- mask stage: [BPC, L] row layout ----
            # first1 = min(where(ids==1, iota, L)), same for 2
            def first_idx(marker: int):
                t = pp.tile([BPC, L], F32, tag=f"t{marker}")
                nc.vector.memset(t[:], float(L))
                ism = pp.tile([BPC, L], I32, tag=f"is{marker}")
                nc.vector.tensor_scalar(
                    out=ism[:], in0=ids_sb[:], scalar1=marker, scalar2=None,
                    op0=Alu.is_equal,
                )
                nc.vector.copy_predicated(t[:], ism[:], iota_sb[:])
                first = pp.tile([BPC, 1], F32, tag=f"first{marker}")
                nc.vector.tensor_reduce(first[:], t[:], axis=X, op=Alu.min)
                return first

            first1 = first_idx(1)
            first2 = first_idx(2)

            g1 = pp.tile([BPC, L], F32)
            nc.vector.tensor_scalar(
                out=g1[:], in0=iota_sb[:], scalar1=first1[:, 0:1], scalar2=None,
                op0=Alu.is_gt,
            )
            mask = pp.tile([BPC, L], F32)
            nc.vector.tensor_scalar(
                out=mask[:], in0=iota_sb[:], scalar1=first2[:, 0:1], scalar2=None,
                op0=Alu.is_lt,
            )
            nc.vector.tensor_mul(mask[:], mask[:], g1[:])

            hs = pp.tile([BPC, 1], F32)
            nc.vector.reduce_max(hs[:], mask[:], axis=X)
            # row 0 contributes CLS exactly when the span is empty
            nc.vector.tensor_scalar(
                out=mask[:, 0:1], in0=hs[:], scalar1=-1.0, scalar2=1.0,
                op0=Alu.mult, op1=Alu.add,
            )

            # transpose mask [BPC, L] -> maskT [128, KL*BPC] (col = k*BPC + b)
            maskT = pp.tile([128, KL * BPC], F32)
            for k in range(KL):
                tp = spsum.tile([128, BPC], F32, tag="small")
                nc.tensor.transpose(
                    tp[:], mask[:, bass.ts(k, 128)], ident_sb[0:BPC, 0:BPC]
                )
                nc.vector.tensor_copy(maskT[:, bass.ts(k, BPC)], tp[:])
            biasT = pp.tile([128, KL * BPC], F32)
            nc.vector.tensor_scalar(
                out=biasT[:], in0=maskT[:], scalar1=BIG, scalar2=BIG,
                op0=Alu.mult, op1=Alu.subtract,
            )

            out_all = pp.tile([128, BPC * JD], F32)

            # ---- main streaming loop ----
            for b in range(BPC):
                x = xpool.tile([128, KL, D], F32, tag="x")
                nc.sync.dma_start(
                    out=x[:], in_=va[b].rearrange("(k p) d -> p k d", p=128)
                )

                # masked copy on ScalarE: m*x + (m-1)*BIG
                xm = mpool.tile([128, KL, D], F32, tag="xm")
                for k in range(KL):
                    col = k * BPC + b
                    nc.scalar.activation(
                        xm[:, k, :], x[:, k, :], Act.Identity,
                        bias=biasT[:, col : col + 1],
                        scale=maskT[:, col : col + 1],
                    )

                # max over the 4 L-tiles -> r [128, D]
                t01 = rpool.tile([128, D], F32, tag="t01")
                nc.vector.tensor_max(t01[:], xm[:, 0, :], xm[:, 1, :])
                t23 = rpool.tile([128, D], F32, tag="t23")
                nc.vector.tensor_max(t23[:], xm[:, 2, :], xm[:, 3, :])
                r = rpool.tile([128, D], F32, tag="r")
                nc.vector.tensor_max(r[:], t01[:], t23[:])

                # cross-partition max via PE transpose + free-dim reduce
                v = vpool.tile([128, JD], F32, tag="v")
                for j in range(JD):
                    tp = ppool.tile([128, 128], F32, tag="tr")
                    nc.tensor.transpose(tp[:], r[:, bass.ts(j, 128)], ident_sb[:])
                    nc.vector.reduce_max(v[:, j : j + 1], tp[:], axis=X)

                # out = cls + mu * vec
                tmp = vpool.tile([128, JD], F32, tag="tmp")
                nc.vector.tensor_scalar_mul(tmp[:], v[:], mu_col[:, 0:1])
                nc.vector.tensor_add(
                    out_all[:, bass.ts(b, JD)], tmp[:], cls_all[:, b, :]
                )

            # ---- store: transpose [128, BPC*JD] -> [BPC*JD, 128] = out ----
            oT_ps = spsum.tile([BPC * JD, 128], F32, tag="small")
            nc.tensor.transpose(oT_ps[:], out_all[:], ident_sb[:])
            oT = vpool.tile([BPC * JD, 128], F32, tag="oT")
            nc.vector.tensor_copy(oT[:], oT_ps[:])
            nc.sync.dma_start(
                out=out.rearrange("b (c p) -> (b c) p", p=128), in_=oT[:]
            )

    nc.compile()
    return nc


def make_const_inputs():
    iota = np.broadcast_to(
        np.arange(L, dtype=np.float32)[None, :], (BPC, L)
    ).copy()
    ident = np.eye(128, dtype=np.float32)
    return iota, ident


def make_in_maps(vector_all, ids, mu):
    va = np.ascontiguousarray(np.asarray(vector_all, dtype=np.float32))
    ids = np.ascontiguousarray(np.asarray(ids, dtype=np.int32))
    mu_col = np.full((128, 1), np.asarray(mu, dtype=np.float32).reshape(-1)[0],
                     dtype=np.float32)
    iota, ident = make_const_inputs()
    in_maps = []
    for c in range(NCORES):
        in_maps.append(
            {
                "vector_all": va[c * BPC : (c + 1) * BPC],
                "ids": ids[c * BPC : (c + 1) * BPC],
                "mu": mu_col,
                "iota": iota,
                "identity": ident,
            }
        )
    return in_maps


def run(vector_all, ids, mu, trace=False):
    """Returns (out [B, D] f32, BassKernelResults)."""
    nc = build_bass()
    in_maps = make_in_maps(vector_all, ids, mu)
    res = run_bass_kernel_spmd(nc, in_maps, list(range(NCORES)), trace=trace)
    out = np.concatenate(
        [res.results[c]["out"] for c in range(NCORES)], axis=0
    ).astype(np.float32)
    return out, res


def kernel(**inputs) -> np.ndarray:
    out, _ = run(inputs["vector_all"], inputs["ids"], inputs["mu"])
    return out


# revision 12
# speedup vs baseline: 1.0176x; 1.0176x over previous
"""Trainium2 Bass kernel for nn_BiEncoder_63024350101542 (segment_reduce).

Computes, per batch row b of vector_all [B=64, L=512, D=1024]:
    mask[b,j] = (j > first_idx(ids[b]==1)) & (j < first_idx(ids[b]==2))
    span_max  = max over masked rows (fallback: CLS row 0 when mask empty)
    out[b]    = cls + mu * span_max

Sharding: pure data parallelism over the batch dim — 8 batches per
NeuronCore across 8 cores. Each core streams its 16 MiB shard of
vector_all once (memory-bound), doing the masked max on-chip.

Note: every PE (transpose) instruction must carry at most one semaphore
wait — walrus rejects matmuls with multiple embedded waits. All PE
inputs are therefore produced by the vector engine (single DVE sem).
"""

import os
import sys

import numpy as np

for _p in ("/root/.axon_site/_ro/trn_rl_repo", "/opt/trn_rl_repo"):
    if _p not in sys.path and os.path.isdir(_p):
        sys.path.append(_p)

import concourse.bacc as bacc
import concourse.bass as bass
import concourse.mybir as mybir
import concourse.tile as tile
from concourse.bass_utils import run_bass_kernel_spmd

F32 = mybir.dt.float32
I32 = mybir.dt.int32
X = mybir.AxisListType.X
Alu = mybir.AluOpType
Act = mybir.ActivationFunctionType

B, L, D = 64, 512, 1024
NCORES = 8
BPC = B // NCORES          # batches per core
KL = L // 128              # L-tiles per batch (4)
JD = D // 128              # d-blocks (8)
BIG = 1.0e30


def build_bass():
    nc = bacc.Bacc("TRN2", target_bir_lowering=False, debug=False)

    va = nc.dram_tensor("vector_all", [BPC, L, D], F32, kind="ExternalInput").ap()
    ids = nc.dram_tensor("ids", [BPC, L], I32, kind="ExternalInput").ap()
    mu = nc.dram_tensor("mu", [128, 1], F32, kind="ExternalInput").ap()
    iota = nc.dram_tensor("iota", [BPC, L], F32, kind="ExternalInput").ap()
    ident = nc.dram_tensor("identity", [128, 128], F32, kind="ExternalInput").ap()
    out = nc.dram_tensor("out", [BPC, D], F32, kind="ExternalOutput").ap()

    with tile.TileContext(nc) as tc:
        with (
            tc.tile_pool(name="persist", bufs=1) as pp,
            tc.tile_pool(name="xin", bufs=3) as xpool,
            tc.tile_pool(name="masked", bufs=2) as mpool,
            tc.tile_pool(name="red", bufs=2) as rpool,
            tc.tile_pool(name="vout", bufs=2) as vpool,
            tc.tile_pool(name="tr", bufs=2, space="PSUM") as ppool,
            tc.tile_pool(name="clsp", bufs=2, space="PSUM") as clspool,
            tc.tile_pool(name="smallp", bufs=2, space="PSUM") as spsum,
        ):
            # ---- constants / inputs for the mask stage ----
            ident_sb = pp.tile([128, 128], F32)
            nc.sync.dma_start(out=ident_sb[:], in_=ident)
            mu_col = pp.tile([128, 1], F32)
            nc.sync.dma_start(out=mu_col[:], in_=mu)
            ids_sb = pp.tile([BPC, L], I32)
            nc.sync.dma_start(out=ids_sb[:], in_=ids)
            iota_sb = pp.tile([BPC, L], F32)
            nc.sync.dma_start(out=iota_sb[:], in_=iota)

            # ---- mask stage: [BPC, L] row layout ----
            # first1 = min(where(ids==1, iota, L)), same for 2
            def first_idx(marker: int):
                t = pp.tile([BPC, L], F32, tag=f"t{marker}")
                nc.vector.memset(t[:], float(L))
                ism = pp.tile([BPC, L], I32, tag=f"is{marker}")
                nc.vector.tensor_scalar(
                    out=ism[:], in0=ids_sb[:], scalar1=marker, scalar2=None,
                    op0=Alu.is_equal,
                )
                nc.vector.copy_predicated(t[:], ism[:], iota_sb[:])
                first = pp.tile([BPC, 1], F32, tag=f"first{marker}")
                nc.vector.tensor_reduce(first[:], t[:], axis=X, op=Alu.min)
                return first

            first1 = first_idx(1)
            first2 = first_idx(2)

            g1 = pp.tile([BPC, L], F32)
            nc.vector.tensor_scalar(
                out=g1[:], in0=iota_sb[:], scalar1=first1[:, 0:1], scalar2=None,
                op0=Alu.is_gt,
            )
            mask = pp.tile([BPC, L], F32)
            nc.vector.tensor_scalar(
                out=mask[:], in0=iota_sb[:], scalar1=first2[:, 0:1], scalar2=None,
                op0=Alu.is_lt,
            )
            nc.vector.tensor_mul(mask[:], mask[:], g1[:])

            hs = pp.tile([BPC, 1], F32)
            nc.vector.reduce_max(hs[:], mask[:], axis=X)
            # row 0 contributes CLS exactly when the span is empty
            nc.vector.tensor_scalar(
                out=mask[:, 0:1], in0=hs[:], scalar1=-1.0, scalar2=1.0,
                op0=Alu.mult, op1=Alu.add,
            )

            # transpose mask [BPC, L] -> maskT [128, KL*BPC] (col = k*BPC + b)
            maskT = pp.tile([128, KL * BPC], F32)
            for k in range(KL):
                tp = spsum.tile([128, BPC], F32, tag="small")
                nc.tensor.transpose(
                    tp[:], mask[:, bass.ts(k, 128)], ident_sb[0:BPC, 0:BPC]
                )
                nc.vector.tensor_copy(maskT[:, bass.ts(k, BPC)], tp[:])
            biasT = pp.tile([128, KL * BPC], F32)
            nc.vector.tensor_scalar(
                out=biasT[:], in0=maskT[:], scalar1=BIG, scalar2=BIG,
                op0=Alu.mult, op1=Alu.subtract,
            )

            out_all = pp.tile([128, BPC * JD], F32)

            # ---- main streaming loop ----
            for b in range(BPC):
                x = xpool.tile([128, KL, D], F32, tag="x")
                # alternate the two HWDGE rings so fixed DMA costs overlap
                dma_eng = nc.sync if b % 2 == 0 else nc.scalar
                dma_eng.dma_start(
                    out=x[:], in_=va[b].rearrange("(k p) d -> p k d", p=128)
                )

                # cls (row 0) into column layout via tiny K=1 matmuls
                cls_ps = clspool.tile([128, JD], F32, tag="cls")
                for j in range(JD):
                    nc.tensor.matmul(
                        cls_ps[:, j : j + 1],
                        lhsT=x[0:1, 0, bass.ts(j, 128)],
                        rhs=ident_sb[0:1, 0:1],
                    )

                # masked copy on ScalarE: m*x + (m-1)*BIG
                xm = mpool.tile([128, KL, D], F32, tag="xm")
                for k in range(KL):
                    col = k * BPC + b
                    nc.scalar.activation(
                        xm[:, k, :], x[:, k, :], Act.Identity,
                        bias=biasT[:, col : col + 1],
                        scale=maskT[:, col : col + 1],
                    )

                # max over the 4 L-tiles -> r [128, D]
                t01 = rpool.tile([128, D], F32, tag="t01")
                nc.vector.tensor_max(t01[:], xm[:, 0, :], xm[:, 1, :])
                t23 = rpool.tile([128, D], F32, tag="t23")
                nc.vector.tensor_max(t23[:], xm[:, 2, :], xm[:, 3, :])
                r = rpool.tile([128, D], F32, tag="r")
                nc.vector.tensor_max(r[:], t01[:], t23[:])

                # cross-partition max via PE transposes + one fused reduce
                p3 = ppool.tile([128, JD, 128], F32, tag="tr")
                for j in range(JD):
                    nc.tensor.transpose(
                        p3[:, j, :], r[:, bass.ts(j, 128)], ident_sb[:]
                    )
                v = vpool.tile([128, JD], F32, tag="v")
                nc.vector.tensor_reduce(v[:], p3[:], axis=X, op=Alu.max)

                # out = cls + mu * vec  (single fused DVE op)
                nc.vector.scalar_tensor_tensor(
                    out=out_all[:, bass.ts(b, JD)],
                    in0=v[:], scalar=mu_col[:, 0:1], in1=cls_ps[:],
                    op0=Alu.mult, op1=Alu.add,
                )

            # ---- store: transpose [128, BPC*JD] -> [BPC*JD, 128] = out ----
            oT_ps = spsum.tile([BPC * JD, 128], F32, tag="small")
            nc.tensor.transpose(oT_ps[:], out_all[:], ident_sb[:])
            oT = vpool.tile([BPC * JD, 128], F32, tag="oT")
            nc.vector.tensor_copy(oT[:], oT_ps[:])
            nc.sync.dma_start(
                out=out.rearrange("b (c p) -> (b c) p", p=128), in_=oT[:]
            )

    nc.compile()
    return nc


def make_const_inputs():
    iota = np.broadcast_to(
        np.arange(L, dtype=np.float32)[None, :], (BPC, L)
    ).copy()
    ident = np.eye(128, dtype=np.float32)
    return iota, ident


def make_in_maps(vector_all, ids, mu):
    va = np.ascontiguousarray(np.asarray(vector_all, dtype=np.float32))
    ids = np.ascontiguousarray(np.asarray(ids, dtype=np.int32))
    mu_col = np.full((128, 1), np.asarray(mu, dtype=np.float32).reshape(-1)[0],
                     dtype=np.float32)
    iota, ident = make_const_inputs()
    in_maps = []
    for c in range(NCORES):
        in_maps.append(
            {
                "vector_all": va[c * BPC : (c + 1) * BPC],
                "ids": ids[c * BPC : (c + 1) * BPC],
                "mu": mu_col,
                "iota": iota,
                "identity": ident,
            }
        )
    return in_maps


def run(vector_all, ids, mu, trace=False):
    """Returns (out [B, D] f32, BassKernelResults)."""
    nc = build_bass()
    in_maps = make_in_maps(vector_all, ids, mu)
    res = run_bass_kernel_spmd(nc, in_maps, list(range(NCORES)), trace=trace)
    out = np.concatenate(
        [res.results[c]["out"] for c in range(NCORES)], axis=0
    ).astype(np.float32)
    return out, res


def kernel(**inputs) -> np.ndarray:
    out, _ = run(inputs["vector_all"], inputs["ids"], inputs["mu"])
    return out


# revision 22
# speedup vs baseline: 1.0232x; 1.0055x over previous
"""Trainium2 Bass kernel for nn_BiEncoder_63024350101542 (segment_reduce).

Computes, per batch row b of vector_all [B=64, L=512, D=1024]:
    mask[b,j] = (j > first_idx(ids[b]==1)) & (j < first_idx(ids[b]==2))
    span_max  = max over masked rows (fallback: CLS row 0 when mask empty)
    out[b]    = cls + mu * span_max

Sharding: pure data parallelism over the batch dim — 8 batches per
NeuronCore across 8 cores. Each core streams its 16 MiB shard of
vector_all once (memory-bound), doing the masked max on-chip.

Note: every PE (transpose) instruction must carry at most one semaphore
wait — walrus rejects matmuls with multiple embedded waits. All PE
inputs are therefore produced by the vector engine (single DVE sem).
"""

import os
import sys

import numpy as np

for _p in ("/root/.axon_site/_ro/trn_rl_repo", "/opt/trn_rl_repo"):
    if _p not in sys.path and os.path.isdir(_p):
        sys.path.append(_p)

import concourse.bacc as bacc
import concourse.bass as bass
import concourse.mybir as mybir
import concourse.tile as tile
from concourse.bass_utils import run_bass_kernel_spmd

F32 = mybir.dt.float32
I32 = mybir.dt.int32
X = mybir.AxisListType.X
Alu = mybir.AluOpType
Act = mybir.ActivationFunctionType

B, L, D = 64, 512, 1024
NCORES = 8
BPC = B // NCORES          # batches per core
KL = L // 128              # L-tiles per batch (4)
JD = D // 128              # d-blocks (8)
BIG = 1.0e30


def build_bass():
    nc = bacc.Bacc("TRN2", target_bir_lowering=False, debug=False)

    va = nc.dram_tensor("vector_all", [BPC, L, D], F32, kind="ExternalInput").ap()
    ids = nc.dram_tensor("ids", [BPC, L], I32, kind="ExternalInput").ap()
    mu = nc.dram_tensor("mu", [128, 1], F32, kind="ExternalInput").ap()
    iota = nc.dram_tensor("iota", [BPC, L], F32, kind="ExternalInput").ap()
    iotap = nc.dram_tensor("iotap", [128, KL], F32, kind="ExternalInput").ap()
    ident = nc.dram_tensor("identity", [128, 128], F32, kind="ExternalInput").ap()
    out = nc.dram_tensor("out", [BPC, D], F32, kind="ExternalOutput").ap()

    with tile.TileContext(nc) as tc:
        with (
            tc.tile_pool(name="persist", bufs=1) as pp,
            tc.tile_pool(name="xin", bufs=3) as xpool,
            tc.tile_pool(name="masked", bufs=2) as mpool,
            tc.tile_pool(name="red", bufs=2) as rpool,
            tc.tile_pool(name="vout", bufs=2) as vpool,
            tc.tile_pool(name="tr", bufs=2, space="PSUM") as ppool,
            tc.tile_pool(name="clsp", bufs=2, space="PSUM") as clspool,
            tc.tile_pool(name="smallp", bufs=1, space="PSUM") as spsum,
        ):
            # ---- constants / inputs for the mask stage ----
            ident_sb = pp.tile([128, 128], F32)
            nc.sync.dma_start(out=ident_sb[:], in_=ident)
            mu_col = pp.tile([128, 1], F32)
            nc.sync.dma_start(out=mu_col[:], in_=mu)
            ids_sb = pp.tile([BPC, L], I32)
            nc.sync.dma_start(out=ids_sb[:], in_=ids)
            iota_sb = pp.tile([BPC, L], F32)
            nc.sync.dma_start(out=iota_sb[:], in_=iota)
            iotap_sb = pp.tile([128, KL], F32)
            nc.sync.dma_start(out=iotap_sb[:], in_=iotap)
            ones_row = pp.tile([1, 128], F32)
            nc.vector.memset(ones_row[:], 1.0)

            # ---- mask stage ----
            # fs[:, 0] = first1, fs[:, 1] = first2, fs[:, 2] = has_span
            fs = pp.tile([BPC, 3], F32)

            def first_idx(marker: int, col: int):
                t = pp.tile([BPC, L], F32, tag=f"t{marker}")
                nc.vector.memset(t[:], float(L))
                ism = pp.tile([BPC, L], I32, tag=f"is{marker}")
                nc.vector.tensor_scalar(
                    out=ism[:], in0=ids_sb[:], scalar1=marker, scalar2=None,
                    op0=Alu.is_equal,
                )
                nc.vector.copy_predicated(t[:], ism[:], iota_sb[:])
                nc.vector.tensor_reduce(
                    fs[:, col : col + 1], t[:], axis=X, op=Alu.min
                )

            first_idx(1, 0)
            first_idx(2, 1)
            # has_span = (first1 + 1 < first2)
            f1p1 = pp.tile([BPC, 1], F32)
            nc.vector.tensor_scalar_add(f1p1[:], fs[:, 0:1], 1.0)
            nc.vector.tensor_tensor(
                out=fs[:, 2:3], in0=f1p1[:], in1=fs[:, 1:2], op=Alu.is_lt
            )

            # transpose each column of fs to a [1, BPC] row at partition 0
            fsT = pp.tile([1, 3, BPC], F32)
            for c in range(3):
                rT = spsum.tile([1, BPC], F32, tag="small")
                nc.tensor.transpose(
                    rT[:], fs[:, c : c + 1], ident_sb[0:BPC, 0:BPC]
                )
                nc.vector.tensor_copy(fsT[:, c, :], rT[:])

            # broadcast first1/first2 across partitions: [128, 2, BPC]
            f12r_ps = spsum.tile([128, 2, BPC], F32, tag="bc")
            nc.tensor.matmul(f12r_ps[:], lhsT=ones_row[:], rhs=fsT[:, 0:2, :])
            f1r_ps = f12r_ps[:, 0, :]
            f2r_ps = f12r_ps[:, 1, :]

            # maskT[p, k*BPC+b] = (4p+k > first1[b]) & (4p+k < first2[b])
            maskT = pp.tile([128, KL * BPC], F32)
            for k in range(KL):
                ga = pp.tile([128, BPC], F32, tag="ga")
                nc.vector.tensor_scalar(
                    out=ga[:], in0=f1r_ps, scalar1=iotap_sb[:, k : k + 1],
                    scalar2=None, op0=Alu.is_lt,
                )
                gb = pp.tile([128, BPC], F32, tag="gb")
                nc.vector.tensor_scalar(
                    out=gb[:], in0=f2r_ps, scalar1=iotap_sb[:, k : k + 1],
                    scalar2=None, op0=Alu.is_gt,
                )
                nc.vector.tensor_mul(maskT[:, bass.ts(k, BPC)], ga[:], gb[:])
            # row 0 (l = 0: p=0, k=0) contributes CLS exactly when span empty
            nc.vector.tensor_scalar(
                out=maskT[0:1, 0:BPC], in0=fsT[:, 2, :], scalar1=-1.0, scalar2=1.0,
                op0=Alu.mult, op1=Alu.add,
            )
            biasT = pp.tile([128, KL * BPC], F32)
            nc.vector.tensor_scalar(
                out=biasT[:], in0=maskT[:], scalar1=BIG, scalar2=BIG,
                op0=Alu.mult, op1=Alu.subtract,
            )

            out_all = pp.tile([128, BPC * JD], F32)

            # ---- main streaming loop ----
            for b in range(BPC):
                x = xpool.tile([128, KL, D], F32, tag="x")
                # alternate the two HWDGE rings so fixed DMA costs overlap
                dma_eng = nc.sync if b % 2 == 0 else nc.scalar
                # 16 KiB contiguous per partition: l = 4p + k
                dma_eng.dma_start(
                    out=x[:], in_=va[b].rearrange("(p k) d -> p k d", k=KL)
                )

                # cls (row 0) into column layout via tiny K=1 matmuls
                cls_ps = clspool.tile([128, JD], F32, tag="cls")
                for j in range(JD):
                    nc.tensor.matmul(
                        cls_ps[:, j : j + 1],
                        lhsT=x[0:1, 0, bass.ts(j, 128)],
                        rhs=ident_sb[0:1, 0:1],
                    )

                # masked copy on ScalarE: m*x + (m-1)*BIG
                xm = mpool.tile([128, KL, D], F32, tag="xm")
                for k in range(KL):
                    col = k * BPC + b
                    nc.scalar.activation(
                        xm[:, k, :], x[:, k, :], Act.Identity,
                        bias=biasT[:, col : col + 1],
                        scale=maskT[:, col : col + 1],
                    )

                # max over the 4 L-tiles -> r [128, D]
                t01 = rpool.tile([128, D], F32, tag="t01")
                nc.vector.tensor_max(t01[:], xm[:, 0, :], xm[:, 1, :])
                t23 = rpool.tile([128, D], F32, tag="t23")
                nc.vector.tensor_max(t23[:], xm[:, 2, :], xm[:, 3, :])
                r = rpool.tile([128, D], F32, tag="r")
                nc.vector.tensor_max(r[:], t01[:], t23[:])

                # cross-partition max via PE transposes + one fused reduce
                p3 = ppool.tile([128, JD, 128], F32, tag="tr")
                for j in range(JD):
                    nc.tensor.transpose(
                        p3[:, j, :], r[:, bass.ts(j, 128)], ident_sb[:]
                    )
                v = vpool.tile([128, JD], F32, tag="v")
                nc.vector.tensor_reduce(v[:], p3[:], axis=X, op=Alu.max)

                # out = cls + mu * vec  (single fused DVE op)
                nc.vector.scalar_tensor_tensor(
                    out=out_all[:, bass.ts(b, JD)],
                    in0=v[:], scalar=mu_col[:, 0:1], in1=cls_ps[:],
                    op0=Alu.mult, op1=Alu.add,
                )

            # ---- store: transpose [128, BPC*JD] -> [BPC*JD, 128] = out ----
            oT_ps = spsum.tile([BPC * JD, 128], F32, tag="small")
            nc.tensor.transpose(oT_ps[:], out_all[:], ident_sb[:])
            oT = vpool.tile([BPC * JD, 128], F32, tag="oT")
            nc.vector.tensor_copy(oT[:], oT_ps[:])
            nc.sync.dma_start(
                out=out.rearrange("b (c p) -> (b c) p", p=128), in_=oT[:]
            )

    nc.compile()
    return nc


def make_const_inputs():
    iota = np.broadcast_to(
        np.arange(L, dtype=np.float32)[None, :], (BPC, L)
    ).copy()
    # iotap[p, k] = l = 4p + k (row index held by partition p, col group k)
    iotap = (
        np.arange(128, dtype=np.float32)[:, None] * KL
        + np.arange(KL, dtype=np.float32)[None, :]
    )
    ident = np.eye(128, dtype=np.float32)
    return iota, iotap, ident


def make_in_maps(vector_all, ids, mu):
    va = np.ascontiguousarray(np.asarray(vector_all, dtype=np.float32))
    ids = np.ascontiguousarray(np.asarray(ids, dtype=np.int32))
    mu_col = np.full((128, 1), np.asarray(mu, dtype=np.float32).reshape(-1)[0],
                     dtype=np.float32)
    iota, iotap, ident = make_const_inputs()
    in_maps = []
    for c in range(NCORES):
        in_maps.append(
            {
                "vector_all": va[c * BPC : (c + 1) * BPC],
                "ids": ids[c * BPC : (c + 1) * BPC],
                "mu": mu_col,
                "iota": iota,
                "iotap": iotap,
                "identity": ident,
            }
        )
    return in_maps


def run(vector_all, ids, mu, trace=False):
    """Returns (out [B, D] f32, BassKernelResults)."""
    nc = build_bass()
    in_maps = make_in_maps(vector_all, ids, mu)
    res = run_bass_kernel_spmd(nc, in_maps, list(range(NCORES)), trace=trace)
    out = np.concatenate(
        [res.results[c]["out"] for c in range(NCORES)], axis=0
    ).astype(np.float32)
    return out, res


def kernel(**inputs) -> np.ndarray:
    out, _ = run(inputs["vector_all"], inputs["ids"], inputs["mu"])
    return out
